# revision 1
# baseline (speedup 1.0000x reference)
"""Trainium2 Bass kernel for GCN(BN->conv1->relu->conv2->relu)->MLP3->log_softmax.

Strategy (8 NeuronCores, graph/data parallel per sharding hint):
  - Nodes sharded contiguously: core c owns rows [c*NLOC, (c+1)*NLOC).
  - Edges sharded by destination core; within a core, grouped by 128-node
    destination blocks and chunked into groups of 128 edges.
  - Per chunk: indirect-DMA row gather of the source features (dma_gather),
    one-hot selection matrix S built on DVE (iota == dst_local) * norm, and a
    PE matmul  aggT += msg^T @ S  accumulating into PSUM per dst block.
    This realizes the scatter-add as dense matmuls (feat-major output).
  - BatchNorm is folded into conv1: z = x*a + bvec  =>
    (A_hat z) W1 = (A_hat x)(diag(a) W1) + r x (bvec W1),  r = A_hat 1.
    Stats (sum x, sum x^2) are computed shard-local + tiny AllReduce.
  - conv1 output is transposed to node-major and AllGathered so every core
    holds the full gather table for conv2 (halo exchange, maximal halo).
  - MLP (no nonlinearity between layers) stays feat-major; final transpose +
    log-softmax per 128-node tile; each core writes its [NLOC, 40] slice.

The gather index payload is int16 (dma_gather ISA), so sources are split into
two streams: lo (src < 32768) and hi (src >= 32768, index rebased, table AP
offset by 32768 rows). Chunk counts per (block, stream) are padded to the max
across the 8 cores so a single SPMD program serves all cores.
"""
import sys

sys.path.insert(0, "/opt/trn_rl_repo")
from contextlib import ExitStack

import numpy as np

import concourse.bacc as bacc
import concourse.bass as bass
import concourse.tile as tile
from concourse import mybir
from concourse.bass_utils import run_bass_kernel_spmd

P = 128
NCORES = 8
LO_LIM = 32768
G_LO = 8    # chunks per gather instruction (HW dma_gather caps ~1024 idxs)
G_HI = 8
EPS = 1e-5
F32 = mybir.dt.float32
I16 = mybir.dt.int16
AF = mybir.ActivationFunctionType
ALU = mybir.AluOpType


# ---------------------------------------------------------------- host plan

def _wrap16(idx_i16):
    """dma_gather index layout: element i at [i % 16, i // 16], replicated
    across the 8 Q7 core stripes (partitions 16k..16k+15)."""
    n = idx_i16.shape[0]
    assert n % 16 == 0
    buf = np.zeros((P, n // 16), dtype=np.int16)
    tiledview = idx_i16.reshape(n // 16, 16).T
    for g in range(8):
        buf[g * 16 : (g + 1) * 16, :] = tiledview
    return buf


def _plan(edge_index, N):
    NLOC = N // NCORES
    NBLK = (NLOC + P - 1) // P
    src = edge_index[0].astype(np.int64)
    dst = edge_index[1].astype(np.int64)
    E = src.shape[0]

    deg = (np.bincount(dst, minlength=N) + 1.0).astype(np.float64)
    dinv = (1.0 / np.sqrt(deg)).astype(np.float32)
    s_acc = np.bincount(dst, weights=dinv[src].astype(np.float64), minlength=N)
    r_full = (dinv * (s_acc.astype(np.float32) + dinv)).astype(np.float32)

    allsrc = np.concatenate([src, np.arange(N, dtype=np.int64)])
    alldst = np.concatenate([dst, np.arange(N, dtype=np.int64)])
    allnorm = (dinv[allsrc] * dinv[alldst]).astype(np.float32)

    core = alldst // NLOC
    blk = (alldst % NLOC) // P
    hi = (allsrc >= LO_LIM).astype(np.int64)
    key = (core * NBLK + blk) * 2 + hi
    order = np.argsort(key, kind="stable")
    ks, ss, dd, nn = key[order], allsrc[order], alldst[order], allnorm[order]
    counts = np.bincount(ks, minlength=NCORES * NBLK * 2).reshape(NCORES, NBLK, 2)
    starts = np.zeros(NCORES * NBLK * 2 + 1, dtype=np.int64)
    np.cumsum(counts.reshape(-1), out=starts[1:])

    nch = -(-counts // P)                        # ceil chunks per (core, blk, stream)
    nlo = nch[:, :, 0].max(axis=0)               # shared across cores
    nhi = nch[:, :, 1].max(axis=0)
    Clo, Chi = int(nlo.sum()), int(nhi.sum())
    Ctot = Clo + Chi
    has_hi = Chi > 0 and N > LO_LIM

    cores = []
    for c in range(NCORES):
        lo_idx = np.zeros(Clo * P, dtype=np.int16)
        hi_idx = np.zeros(max(Chi, 1) * P, dtype=np.int16)
        dl = np.zeros((P, Ctot), dtype=np.float32)
        nm = np.zeros((P, Ctot), dtype=np.float32)
        t = 0
        lo_p = 0
        hi_p = 0
        for b in range(NBLK):
            base = c * NLOC + b * P
            for stream in (0, 1):
                k = (c * NBLK + b) * 2 + stream
                e0, e1 = starts[k], starts[k + 1]
                cnt = e1 - e0
                nchunks = nlo[b] if stream == 0 else nhi[b]
                idx_arr = ss[e0:e1] - (LO_LIM if stream else 0)
                dl_arr = dd[e0:e1] - base
                nm_arr = nn[e0:e1]
                for j in range(nchunks):
                    a0 = j * P
                    a1 = min(a0 + P, cnt)
                    nv = max(a1 - a0, 0)
                    col_dl = np.zeros(P, dtype=np.float32)
                    col_nm = np.zeros(P, dtype=np.float32)
                    chunk_idx = np.zeros(P, dtype=np.int16)
                    if nv > 0:
                        chunk_idx[:nv] = idx_arr[a0:a1].astype(np.int16)
                        col_dl[:nv] = dl_arr[a0:a1].astype(np.float32)
                        col_nm[:nv] = nm_arr[a0:a1]
                    if stream == 0:
                        lo_idx[lo_p * P : (lo_p + 1) * P] = chunk_idx
                        lo_p += 1
                    else:
                        hi_idx[hi_p * P : (hi_p + 1) * P] = chunk_idx
                        hi_p += 1
                    dl[:, t] = col_dl
                    nm[:, t] = col_nm
                    t += 1
        r_row = np.zeros((1, NBLK * P), dtype=np.float32)
        r_row[0, :NLOC] = r_full[c * NLOC : (c + 1) * NLOC]
        cores.append(dict(
            idx_lo=_wrap16(lo_idx),
            idx_hi=_wrap16(hi_idx),
            dl_cols=dl, nm_cols=nm, r_row=r_row,
        ))
    consts = dict(N=N, NLOC=NLOC, NBLK=NBLK, nlo=tuple(int(v) for v in nlo),
                  nhi=tuple(int(v) for v in nhi), Clo=Clo, Chi=Chi, Ctot=Ctot,
                  has_hi=has_hi)
    return consts, cores


# ---------------------------------------------------------------- program

def _ceil_div(a, b):
    return -(-a // b)


def _build(consts, n_classes):
    import os
    PHASES = int(os.environ.get("K_PHASES", "9"))
    N = consts["N"]; NLOC = consts["NLOC"]; NBLK = consts["NBLK"]
    nlo = consts["nlo"]; nhi = consts["nhi"]
    Clo = consts["Clo"]; Chi = consts["Chi"]; Ctot = consts["Ctot"]
    has_hi = consts["has_hi"]
    NBC = NBLK * P              # padded per-core node columns
    NT = _ceil_div(NBC, 512)    # dense 512-wide tiles
    C = n_classes

    nc = bacc.Bacc("TRN2", target_bir_lowering=False, num_devices=NCORES,
                   num_swdge_queues=2)
    x = nc.dram_tensor("x", [N, P], F32, kind="ExternalInput")
    x_slice = nc.dram_tensor("x_slice", [NLOC, P], F32, kind="ExternalInput")
    gammaP = nc.dram_tensor("gammaP", [P, 1], F32, kind="ExternalInput")
    betaP = nc.dram_tensor("betaP", [P, 1], F32, kind="ExternalInput")
    W1 = nc.dram_tensor("W1", [P, P], F32, kind="ExternalInput")
    W2 = nc.dram_tensor("W2", [P, P], F32, kind="ExternalInput")
    Wm1 = nc.dram_tensor("Wm1", [P, P], F32, kind="ExternalInput")
    Wm2 = nc.dram_tensor("Wm2", [P, P], F32, kind="ExternalInput")
    Wm3 = nc.dram_tensor("Wm3", [P, C], F32, kind="ExternalInput")
    b1P = nc.dram_tensor("b1P", [P, 1], F32, kind="ExternalInput")
    b2P = nc.dram_tensor("b2P", [P, 1], F32, kind="ExternalInput")
    bm1P = nc.dram_tensor("bm1P", [P, 1], F32, kind="ExternalInput")
    bm2P = nc.dram_tensor("bm2P", [P, 1], F32, kind="ExternalInput")
    bm3P = nc.dram_tensor("bm3P", [P, 1], F32, kind="ExternalInput")
    iota_row = nc.dram_tensor("iota_row", [1, P], F32, kind="ExternalInput")
    ident = nc.dram_tensor("ident", [P, P], F32, kind="ExternalInput")
    r_row_d = nc.dram_tensor("r_row", [1, NBC], F32, kind="ExternalInput")
    idx_lo_d = nc.dram_tensor("idx_lo", [P, Clo * 8], I16, kind="ExternalInput")
    idx_hi_d = nc.dram_tensor("idx_hi", [P, max(Chi, 1) * 8], I16, kind="ExternalInput")
    dl_d = nc.dram_tensor("dl_cols", [P, Ctot], F32, kind="ExternalInput")
    nm_d = nc.dram_tensor("nm_cols", [P, Ctot], F32, kind="ExternalInput")
    y_out = nc.dram_tensor("y_out", [NLOC, C], F32, kind="ExternalOutput")

    with ExitStack() as ctx:
        tc = ctx.enter_context(tile.TileContext(nc))
        csb = ctx.enter_context(tc.tile_pool(name="const", bufs=1))
        wsb = ctx.enter_context(tc.tile_pool(name="work", bufs=1))
        ssb = ctx.enter_context(tc.tile_pool(name="spool", bufs=8))
        xsb = ctx.enter_context(tc.tile_pool(name="xstat", bufs=3))
        st_lo = ctx.enter_context(tc.tile_pool(name="stlo", bufs=4))
        st_hi = ctx.enter_context(tc.tile_pool(name="sthi", bufs=4))
        osb = ctx.enter_context(tc.tile_pool(name="outp", bufs=4))
        ps_agg = ctx.enter_context(tc.tile_pool(name="psagg", bufs=2, space="PSUM"))
        ps_dn = ctx.enter_context(tc.tile_pool(name="psdn", bufs=2, space="PSUM"))
        ps_tr = ctx.enter_context(tc.tile_pool(name="pstr", bufs=2, space="PSUM"))
        ps_st = ctx.enter_context(tc.tile_pool(name="psst", bufs=1, space="PSUM"))
        dram = ctx.enter_context(tc.tile_pool(name="dram", bufs=1, space="DRAM"))

        def load_const(name, src_ap, shape, dtype=F32):
            t = csb.tile(shape, dtype, tag="c_" + name)
            nc.sync.dma_start(t[:], src_ap)
            return t

        iota_t = load_const("iota", iota_row[:], [1, P])
        ident_t = load_const("ident", ident[:], [P, P])
        gam_t = load_const("gam", gammaP[:], [P, 1])
        bet_t = load_const("bet", betaP[:], [P, 1])
        W1_t = load_const("W1", W1[:], [P, P])
        W2_t = load_const("W2", W2[:], [P, P])
        Wm1_t = load_const("Wm1", Wm1[:], [P, P])
        Wm2_t = load_const("Wm2", Wm2[:], [P, P])
        Wm3_t = load_const("Wm3", Wm3[:], [P, C])
        b1_t = load_const("b1", b1P[:], [P, 1])
        b2_t = load_const("b2", b2P[:], [P, 1])
        bm1_t = load_const("bm1", bm1P[:], [P, 1])
        bm2_t = load_const("bm2", bm2P[:], [P, 1])
        bm3_t = load_const("bm3", bm3P[:], [P, 1])
        r_t = load_const("r", r_row_d[:], [1, NBC])
        dl_t = load_const("dl", dl_d[:], [P, Ctot])
        nm_t = load_const("nm", nm_d[:], [P, Ctot])
        idx_lo_t = load_const("ixlo", idx_lo_d[:], [P, Clo * 8], I16)
        idx_hi_t = load_const("ixhi", idx_hi_d[:], [P, max(Chi, 1) * 8], I16)

        REPS = int(os.environ.get("K_REPS", "1"))
        # iota broadcast [P, P]: ones[1,P]^T @ iota_row
        ones_row = csb.tile([1, P], F32)
        nc.vector.memset(ones_row[:], 1.0)
        iota_ps = ps_tr.tile([P, P], F32, tag="trp", space="PSUM")
        nc.tensor.matmul(out=iota_ps[:], lhsT=ones_row[:], rhs=iota_t[:],
                         start=True, stop=True)
        iota_b = csb.tile([P, P], F32)
        nc.vector.tensor_copy(out=iota_b[:], in_=iota_ps[:])

        REPS = int(os.environ.get("K_REPS", "1"))
        def _pipeline_body():
            # ---------------- phase A: BN stats (shard local + AllReduce) -------
            ones_col = csb.tile([P, 1], F32)
            nc.vector.memset(ones_col[:], 1.0)
            sum_ps = ps_st.tile([1, P], F32, tag="sums", space="PSUM")
            sq_ps = ps_st.tile([1, P], F32, tag="sqs", space="PSUM")
            nfull, rem = divmod(NLOC, P)
            nstat = nfull + (1 if rem else 0)
            for b in range(nstat):
                xt = xsb.tile([P, P], F32, tag="xt")
                nb = P if b < nfull else rem
                if nb < P:
                    nc.vector.memset(xt[:], 0.0)
                nc.sync.dma_start(xt[:nb, :], x_slice[b * P : b * P + nb, :])
                sq = xsb.tile([P, P], F32, tag="xsq")
                nc.scalar.activation(out=sq[:], in_=xt[:], func=AF.Square)
                nc.tensor.matmul(out=sum_ps[:], lhsT=ones_col[:], rhs=xt[:],
                                 start=(b == 0), stop=(b == nstat - 1))
                nc.tensor.matmul(out=sq_ps[:], lhsT=ones_col[:], rhs=sq[:],
                                 start=(b == 0), stop=(b == nstat - 1))
            stats_sb = wsb.tile([1, 2 * P], F32)
            nc.vector.tensor_copy(out=stats_sb[0:1, 0:P], in_=sum_ps[:])
            nc.vector.tensor_copy(out=stats_sb[0:1, P : 2 * P], in_=sq_ps[:])
            NOAR = os.environ.get("K_NOAR", "0") == "1"
            st_in = dram.tile([1, 2 * P], F32)
            st_out = dram.tile([1, 2 * P], F32)
            nc.sync.dma_start(st_in[:], stats_sb[:])
            if not NOAR:
                nc.gpsimd.collective_compute(
                    "AllReduce", ALU.add, replica_groups=[list(range(NCORES))],
                    ins=[st_in[:].opt()], outs=[st_out[:].opt()])
            else:
                nc.sync.dma_start(st_out[:], st_in[:])
            stats_ag = wsb.tile([1, 2 * P], F32)
            nc.sync.dma_start(stats_ag[:], st_out[:])
            stT_ps = ps_tr.tile([P, P], F32, tag="trp", space="PSUM")
            nc.tensor.transpose(out=stT_ps[:, 0:1], in_=stats_ag[0:1, 0:P],
                                identity=ident_t[0:1, 0:1])
            nc.tensor.transpose(out=stT_ps[:, 1:2], in_=stats_ag[0:1, P : 2 * P],
                                identity=ident_t[0:1, 0:1])
            stT = wsb.tile([P, 2], F32)
            nc.vector.tensor_copy(out=stT[:], in_=stT_ps[:, 0:2])
            invN_t = wsb.tile([P, 1], F32)
            nc.vector.memset(invN_t[:], 1.0 / N)
            eps_t = wsb.tile([P, 1], F32)
            nc.vector.memset(eps_t[:], EPS)
            mu = wsb.tile([P, 1], F32)
            nc.vector.tensor_tensor(out=mu[:], in0=stT[:, 0:1], in1=invN_t[:], op=ALU.mult)
            ex2 = wsb.tile([P, 1], F32)
            nc.vector.tensor_tensor(out=ex2[:], in0=stT[:, 1:2], in1=invN_t[:], op=ALU.mult)
            musq = wsb.tile([P, 1], F32)
            nc.vector.tensor_tensor(out=musq[:], in0=mu[:], in1=mu[:], op=ALU.mult)
            var = wsb.tile([P, 1], F32)
            nc.vector.tensor_tensor(out=var[:], in0=ex2[:], in1=musq[:], op=ALU.subtract)
            vare = wsb.tile([P, 1], F32)
            nc.vector.tensor_tensor(out=vare[:], in0=var[:], in1=eps_t[:], op=ALU.add)
            sd = wsb.tile([P, 1], F32)
            nc.scalar.activation(out=sd[:], in_=vare[:], func=AF.Sqrt)
            inv = wsb.tile([P, 1], F32)
            nc.vector.reciprocal(out=inv[:], in_=sd[:])
            a_vec = wsb.tile([P, 1], F32)
            nc.vector.tensor_tensor(out=a_vec[:], in0=inv[:], in1=gam_t[:], op=ALU.mult)
            mua = wsb.tile([P, 1], F32)
            nc.vector.tensor_tensor(out=mua[:], in0=mu[:], in1=a_vec[:], op=ALU.mult)
            bvec = wsb.tile([P, 1], F32)
            nc.vector.tensor_tensor(out=bvec[:], in0=bet_t[:], in1=mua[:], op=ALU.subtract)
            W1p = wsb.tile([P, P], F32)
            nc.vector.tensor_scalar(out=W1p[:], in0=W1_t[:], scalar1=a_vec[:],
                                    scalar2=None, op0=ALU.mult)
            bW1_ps = ps_tr.tile([P, P], F32, tag="trp", space="PSUM")
            nc.tensor.matmul(out=bW1_ps[0:1, :], lhsT=bvec[:], rhs=W1_t[:], start=True, stop=True)
            bW1 = wsb.tile([1, P], F32)
            nc.vector.tensor_copy(out=bW1[:], in_=bW1_ps[0:1, :])

            if PHASES < 2:
                dummy = osb.tile([P, 2], F32, tag="dmy")
                nc.vector.tensor_copy(out=dummy[:], in_=stT[:, 0:2])
                nc.sync.dma_start(y_out[0:P, 0:2], dummy[:])

            # ---------------- conv pass -----------------------------------------
            n_lo_groups = _ceil_div(Clo, G_LO)
            n_hi_groups = _ceil_div(Chi, G_HI) if has_hi else 0

            ABL = set(filter(None, os.environ.get("K_ABL", "").split(",")))

            def run_conv(tbl_lo, tbl_hi, dense_lhsT, bias_t, rank1, out_buf):
                lo_tiles = {}
                hi_tiles = {}
                lo_p = 0
                hi_p = 0
                t = 0
                for b in range(NBLK):
                    nchunks_b = nlo[b] + nhi[b]
                    if nchunks_b == 0:
                        continue
                    agg_ps = None
                    if "nomm" not in ABL:
                        agg_ps = ps_agg.tile([P, P], F32, tag="aggps", space="PSUM")
                    for i in range(nchunks_b):
                        if "nogather" in ABL:
                            msg = iota_b[:]
                            if i < nlo[b]:
                                lo_p += 1
                            else:
                                hi_p += 1
                        elif i < nlo[b]:
                            g, slot = divmod(lo_p, G_LO)
                            if slot == 0:
                                gsz = min(G_LO, Clo - g * G_LO)
                                stile = st_lo.tile([P, gsz, P], F32, tag="stlo")
                                nidx = gsz * P
                                nc.gpsimd.dma_gather(
                                    stile[:], tbl_lo,
                                    idx_lo_t[:, g * G_LO * 8 : (g * G_LO + gsz) * 8],
                                    nidx, nidx, P, queue_num=g % 2)
                                lo_tiles[g] = stile
                            msg = lo_tiles[g][:, slot, :]
                            lo_p += 1
                        else:
                            g, slot = divmod(hi_p, G_HI)
                            if slot == 0:
                                gsz = min(G_HI, Chi - g * G_HI)
                                stile = st_hi.tile([P, gsz, P], F32, tag="sthi")
                                nidx = gsz * P
                                nc.gpsimd.dma_gather(
                                    stile[:], tbl_hi,
                                    idx_hi_t[:, g * G_HI * 8 : (g * G_HI + gsz) * 8],
                                    nidx, nidx, P, queue_num=(g + 1) % 2)
                                hi_tiles[g] = stile
                            msg = hi_tiles[g][:, slot, :]
                            hi_p += 1
                        if "nosbuild" in ABL:
                            S_ap = iota_b[:]
                        else:
                            S = ssb.tile([P, P], F32, tag="S")
                            nc.vector.tensor_scalar(
                                out=S[:], in0=iota_b[:],
                                scalar1=dl_t[:, t : t + 1], scalar2=nm_t[:, t : t + 1],
                                op0=ALU.is_equal, op1=ALU.mult)
                            S_ap = S[:]
                        if "nomm" not in ABL:
                            nc.tensor.matmul(out=agg_ps[:], lhsT=msg, rhs=S_ap,
                                             start=(i == 0), stop=(i == nchunks_b - 1))
                        t += 1
                    nc.scalar.activation(out=out_buf[:, b * P : (b + 1) * P],
                                         in_=agg_ps[:] if agg_ps is not None else iota_b[:],
                                         func=AF.Copy)
                # dense: actT = relu(dense_lhsT.T @ aggT (+ rank1) + bias)
                act_buf = wsb.tile([P, NBC], F32, tag="actT")
                for ti in range(NT):
                    c0 = ti * 512
                    w = min(512, NBC - c0)
                    hps = ps_dn.tile([P, w], F32, tag="hps", space="PSUM")
                    nc.tensor.matmul(out=hps[:], lhsT=dense_lhsT[:], rhs=out_buf[:, c0 : c0 + w],
                                     start=True, stop=(rank1 is None))
                    if rank1 is not None:
                        nc.tensor.matmul(out=hps[:], lhsT=bW1[:], rhs=r_t[:, c0 : c0 + w],
                                         start=False, stop=True)
                    nc.scalar.activation(out=act_buf[:, c0 : c0 + w], in_=hps[:],
                                         func=AF.Relu, bias=bias_t[:])
                return act_buf

            if PHASES >= 2:
                aggX = wsb.tile([P, NBC], F32, tag="aggT")
                a1T = run_conv(x[0:min(LO_LIM, N), :], x[LO_LIM:N, :] if has_hi else None,
                               W1p, b1_t, True, aggX)

            # ---------------- transpose + AllGather -----------------------------
            if PHASES >= 3:
                ag_in = dram.tile([NLOC, P], F32)
                ag_out = dram.tile([N, P], F32)
                for b in range(NBLK):
                    nm_ps = ps_tr.tile([P, P], F32, tag="trp", space="PSUM")
                    nc.tensor.transpose(out=nm_ps[:], in_=a1T[:, b * P : (b + 1) * P],
                                        identity=ident_t[:])
                    nm_sb = osb.tile([P, P], F32, tag="trs")
                    nc.vector.tensor_copy(out=nm_sb[:], in_=nm_ps[:])
                    nb = min(P, NLOC - b * P)
                    nc.sync.dma_start(ag_in[b * P : b * P + nb, :], nm_sb[:nb, :])
                if os.environ.get("K_NOAG", "0") == "1":
                    nc.sync.dma_start(ag_out[0:NLOC, :], ag_in[:])
                else:
                    nc.gpsimd.collective_compute(
                        "AllGather", ALU.bypass, replica_groups=[list(range(NCORES))],
                        ins=[ag_in[:].opt()], outs=[ag_out[:].opt()])

            # ---------------- conv2 ---------------------------------------------
            if PHASES >= 4:
                aggA = wsb.tile([P, NBC], F32, tag="aggT")
                a2T = run_conv(ag_out[0:min(LO_LIM, N), :],
                               ag_out[LO_LIM:N, :] if has_hi else None,
                               W2_t, b2_t, None, aggA)

            # ---------------- MLP (feat-major) ----------------------------------
            def lin(in_buf, w_t, bias_t, width, tagm):
                ob = wsb.tile([P, NBC], F32, tag=tagm)
                for ti in range(NT):
                    c0 = ti * 512
                    w = min(512, NBC - c0)
                    hps = ps_dn.tile([P, w], F32, tag="hps", space="PSUM")
                    nc.tensor.matmul(out=hps[:width, :], lhsT=w_t[:], rhs=in_buf[:, c0 : c0 + w],
                                     start=True, stop=True)
                    nc.vector.tensor_scalar(out=ob[:width, c0 : c0 + w], in0=hps[:width, :],
                                            scalar1=bias_t[:width], scalar2=None, op0=ALU.add)
                return ob

            if PHASES >= 5:
                y1T = lin(a2T, Wm1_t, bm1_t, P, "aggT")
                y2T = lin(y1T, Wm2_t, bm2_t, P, "mlpB")
                y3T = lin(y2T, Wm3_t, bm3_t, C, "aggT")

            # ---------------- log-softmax + output ------------------------------
            if PHASES >= 6:
              for b in range(NBLK):
                ynm_ps_full = ps_tr.tile([P, P], F32, tag="trp", space="PSUM")
                ynm_ps = ynm_ps_full[:, 0:C]
                nc.tensor.transpose(out=ynm_ps, in_=y3T[0:C, b * P : (b + 1) * P],
                                    identity=ident_t[0:C, 0:C])
                m = osb.tile([P, 1], F32, tag="smm")
                nc.vector.reduce_max(out=m[:], in_=ynm_ps, axis=mybir.AxisListType.X)
                tt = osb.tile([P, C], F32, tag="smt")
                nc.vector.tensor_scalar(out=tt[:], in0=ynm_ps, scalar1=m[:],
                                        scalar2=None, op0=ALU.subtract)
                ee = osb.tile([P, C], F32, tag="sme")
                ssum = osb.tile([P, 1], F32, tag="sms")
                nc.scalar.activation(out=ee[:], in_=tt[:], func=AF.Exp, accum_out=ssum[:])
                lns = osb.tile([P, 1], F32, tag="sml")
                nc.scalar.activation(out=lns[:], in_=ssum[:], func=AF.Ln)
                oo = osb.tile([P, C], F32, tag="smo")
                nc.vector.tensor_scalar(out=oo[:], in0=tt[:], scalar1=lns[:],
                                        scalar2=None, op0=ALU.subtract)
                nb = min(P, NLOC - b * P)
                nc.sync.dma_start(y_out[b * P : b * P + nb, :], oo[:nb, :])


        for _rep in range(REPS):
            _pipeline_body()

    nc.compile()
    return nc


# ---------------------------------------------------------------- entry

_CACHE = {}


def kernel(x, edge_index, gamma, beta, W1, b1, W2, b2,
           Wm1, bm1, Wm2, bm2, Wm3, bm3, _trace=False):
    x = np.asarray(x, dtype=np.float32)
    edge_index = np.asarray(edge_index)
    N, F = x.shape
    Cc = Wm3.shape[1]
    NLOC = N // NCORES

    consts, cores = _plan(edge_index, N)
    key = (N, F, Cc) + (consts["nlo"], consts["nhi"])
    if key not in _CACHE:
        _CACHE[key] = _build(consts, Cc)
    nc = _CACHE[key]

    def colvec(v, n=P):
        out = np.zeros((n, 1), dtype=np.float32)
        out[: v.shape[0], 0] = v
        return out

    iota = np.arange(P, dtype=np.float32).reshape(1, P)
    ident = np.eye(P, dtype=np.float32)
    in_maps = []
    for c in range(NCORES):
        cd = cores[c]
        in_maps.append(dict(
            x=x,
            x_slice=np.ascontiguousarray(x[c * NLOC : (c + 1) * NLOC]),
            gammaP=colvec(np.asarray(gamma, np.float32)),
            betaP=colvec(np.asarray(beta, np.float32)),
            W1=np.asarray(W1, np.float32), W2=np.asarray(W2, np.float32),
            Wm1=np.asarray(Wm1, np.float32), Wm2=np.asarray(Wm2, np.float32),
            Wm3=np.asarray(Wm3, np.float32),
            b1P=colvec(np.asarray(b1, np.float32)),
            b2P=colvec(np.asarray(b2, np.float32)),
            bm1P=colvec(np.asarray(bm1, np.float32)),
            bm2P=colvec(np.asarray(bm2, np.float32)),
            bm3P=colvec(np.asarray(bm3, np.float32)),
            iota_row=iota, ident=ident,
            r_row=cd["r_row"], idx_lo=cd["idx_lo"], idx_hi=cd["idx_hi"],
            dl_cols=cd["dl_cols"], nm_cols=cd["nm_cols"],
        ))
    res = run_bass_kernel_spmd(nc, in_maps, list(range(NCORES)), trace=_trace)
    global _last_exec_ns, _last_results
    _last_results = res
    _last_exec_ns = res.exec_time_ns
    return np.concatenate([res.results[c]["y_out"] for c in range(NCORES)], axis=0)

